# revision 1
# baseline (speedup 1.0000x reference)
"""Trainium2 Bass kernel for the fused GNN message-passing block.

Reference computation (per batch b):
    h = silu(x @ W1 + b1) @ W2 + b2                       # [K, C]
    out[q, d, c] = sum_k mask[q,k] * ev[q,k,d] * ef[q,k,c] * h[k,c]

Sharding: data-parallel over (b, q-half) -> 8 cores, each core handles
one b (of 4) and 64 of the 128 q values.  All large tensors carry the
leading b dim; the tiny MLP weights are replicated.

Per-core device program (memory-bound; the ef slice is 8 MiB):
  - compute h via PE matmuls (x transposed on-chip with PE transposes,
    biases folded into the PSUM accumulation as rank-1 matmuls)
  - build w[k, d, q] = (mask * ev)^T via PE transposes + one DVE multiply
  - stream ef in [128(k), 8(q), 256(c)] tiles (1 MiB DMAs), multiply by
    h broadcast over q on DVE, then one tiny matmul per q on PE:
        out[d, c] = sum_k w[k, d, q] * (ef*h)[k, c]
    Four q-outputs are packed into one PSUM bank at partition offsets
    0/32/64/96 via tile_position col-groups so a single ACT copy drains
    four results at once.

The walrus build in this container accepts at most ONE sync wait per
instruction (setupSyncWait in CoreV3GenImpl), while Tile emits one wait
per dependent processor (the mandatory kernel-tail drain alone carries
~12).  _split_multiwaits() post-processes the finalized BIR: for every
instruction with N>1 waits it inserts N-1 single-wait NOPs immediately
before it on the same engine queue.  The sequencer executes waits in
queue order, so waiting serially on preceding NOPs is semantically
identical to the conjunctive multi-wait.
"""

import numpy as np

import concourse.bass as bass
import concourse.mybir as mybir
import concourse.tile as tile
from concourse.bass import ds, ts
from concourse.bass_utils import run_bass_kernel_spmd
from concourse.masks import make_identity

B, Q, K, D, C = 4, 128, 128, 3, 256
N_CORES = 8
QSH = Q // 2  # 64 q rows per core
QB = 8  # q values per ef tile (1 MiB DMA)
NG = QSH // QB
F32 = mybir.dt.float32

_NC_CACHE = {}


def _split_multiwaits(nc):
    """Legalize for the 1-sync-wait-per-instruction walrus: hoist all but
    the last wait of each instruction onto single-wait NOPs placed just
    before it on the same engine queue."""
    n = 0
    for f in nc.m.functions:
        for bb in f.blocks:
            out = []
            for inst in bb.instructions:
                si = inst.sync_info
                if si is not None and si.on_wait and len(si.on_wait) > 1:
                    waits = list(si.on_wait)
                    for w in waits[:-1]:
                        n += 1
                        nop = mybir.InstNoOp(
                            name=f"{inst.name}-wsplit{n}", ins=[], outs=[]
                        )
                        nop.engine = inst.engine
                        nop.sync_info = mybir.SyncInfo(on_wait=[w], on_update=[])
                        out.append(nop)
                    inst.sync_info = mybir.SyncInfo(
                        on_wait=[waits[-1]], on_update=list(si.on_update)
                    )
                out.append(inst)
            bb.instructions = out
    return nc


def _build_nc(split=True):
    nc = bass.Bass()

    ef_d = nc.declare_dram_parameter("efT", [K, QSH, C], F32, isOutput=False)
    evT_d = nc.declare_dram_parameter("evT", [K, D, QSH], F32, isOutput=False)
    maskT_d = nc.declare_dram_parameter("maskT", [K, QSH], F32, isOutput=False)
    xT_d = nc.declare_dram_parameter("xT", [C, K], F32, isOutput=False)
    w1_d = nc.declare_dram_parameter("W1", [C, C], F32, isOutput=False)
    b1_d = nc.declare_dram_parameter("b1", [C], F32, isOutput=False)
    w2_d = nc.declare_dram_parameter("W2", [C, C], F32, isOutput=False)
    b2_d = nc.declare_dram_parameter("b2", [C], F32, isOutput=False)
    out_d = nc.declare_dram_parameter("out", [QSH, D, C], F32, isOutput=True)

    with tile.TileContext(nc) as tc:
        with (
            tc.tile_pool(name="const", bufs=1) as cpool,
            tc.tile_pool(name="efp", bufs=1) as efpool,
            tc.tile_pool(name="outp", bufs=3) as outpool,
            tc.tile_pool(name="pprep", bufs=1, space="PSUM") as pprep,
            tc.tile_pool(name="pout", bufs=5, space="PSUM") as pout,
        ):
            # ---- PE warm-up: ~3.4us of dep-light matmuls flips HAM to 8/8
            # before the MLP chain and main loop need the PE ----
            w_warm = cpool.tile([128, 2 * C], F32)
            nc.vector.memset(w_warm[:], 0.0)
            warm_ps = pout.tile([128, 2 * C], F32, tag="opsum", name="warm_ps")
            for _ in range(6):
                nc.tensor.matmul(
                    warm_ps[:, :C], w_warm[:, :128], w_warm[:, :C], start=True, stop=True
                )

            # ---- constants + ef prefetch, all on the SP queue in need-order
            # (HWDGE lane-count coupling: anything issued after an ef DMA on
            # the same lane waits for that whole 1 MiB transfer) ----
            ef_slots = [
                efpool.tile([K, QB, C], F32, tag=f"ef{g}", name=f"ef{g}")
                for g in range(NG)
            ]
            xT_sb = cpool.tile([128, 2, K], F32)
            nc.sync.dma_start(xT_sb[:], xT_d[:, :].rearrange("(o p) k -> p o k", p=128))
            w1_sb = cpool.tile([128, 2, C], F32)
            nc.sync.dma_start(w1_sb[:], w1_d[:, :].rearrange("(o p) n -> p o n", p=128))
            b1_sb = cpool.tile([1, C], F32)
            nc.sync.dma_start(b1_sb[:], b1_d[:][None])
            w2_sb = cpool.tile([128, 2, C], F32)
            nc.sync.dma_start(w2_sb[:], w2_d[:, :].rearrange("(o p) n -> p o n", p=128))
            b2_sb = cpool.tile([1, C], F32)
            nc.sync.dma_start(b2_sb[:], b2_d[:][None])
            nc.sync.dma_start(ef_slots[0][:], ef_d[:, ts(0, QB), :])
            nc.sync.dma_start(ef_slots[1][:], ef_d[:, ts(1, QB), :])
            evT_sb = cpool.tile([K, D, QSH], F32)
            nc.sync.dma_start(evT_sb[:], evT_d[:, :, :])
            maskT_sb = cpool.tile([K, QSH], F32)
            nc.sync.dma_start(maskT_sb[:], maskT_d[:, :])
            for g in range(2, NG):
                nc.sync.dma_start(ef_slots[g][:], ef_d[:, ts(g, QB), :])
            ones_sb = cpool.tile([1, 128], F32)
            nc.gpsimd.memset(ones_sb[:], 1.0)

            # ---- MLP, fully transpose-free:
            # h1T[d,k] = (W1 slices)^T-style matmuls, silu in [d,k] layout,
            # then h[k,c] directly: lhsT = h1sT (natural), rhs = W2 (natural)
            h1T_ps = [
                pprep.tile([128, 128], F32, tag=f"prep{i}", name=f"h1T{i}")
                for i in range(2)
            ]
            for dh in range(2):
                nc.tensor.matmul(
                    h1T_ps[dh][:], w1_sb[:, 0, ts(dh, 128)], xT_sb[:, 0, :],
                    start=True, stop=False,
                )
                nc.tensor.matmul(
                    h1T_ps[dh][:], w1_sb[:, 1, ts(dh, 128)], xT_sb[:, 1, :],
                    start=False, stop=False,
                )
                nc.tensor.matmul(
                    h1T_ps[dh][:], b1_sb[:, ts(dh, 128)], ones_sb[:],
                    start=False, stop=True,
                )
            sigT_sb = cpool.tile([128, 2, 128], F32)
            h1sT_sb = cpool.tile([128, 2, 128], F32)
            for dh in range(2):
                nc.scalar.activation(
                    sigT_sb[:, dh], h1T_ps[dh][:], mybir.ActivationFunctionType.Sigmoid
                )
                nc.vector.tensor_tensor(
                    h1sT_sb[:, dh], h1T_ps[dh][:], sigT_sb[:, dh], mybir.AluOpType.mult
                )
            h_ps = pprep.tile([128, C], F32, tag="hps", name="h_ps")
            nc.tensor.matmul(h_ps[:], h1sT_sb[:, 0], w2_sb[:, 0], start=True, stop=False)
            nc.tensor.matmul(h_ps[:], h1sT_sb[:, 1], w2_sb[:, 1], start=False, stop=False)
            nc.tensor.matmul(h_ps[:], ones_sb[:], b2_sb[:], start=False, stop=True)
            h_sb = cpool.tile([128, C], F32)
            nc.scalar.copy(out=h_sb[:], in_=h_ps[:])

            # ---- w[k, q, 32] = (mask * ev)^T padded to 32 stationary columns
            # (cols d=3..31 stay zero so each matmul fills its whole 32-row
            # PSUM col-group and the drain copy never reads uninit PSUM) ----
            w_sb = cpool.tile([128, QSH, 32], F32)
            nc.gpsimd.memset(w_sb[:], 0.0)
            for d in range(D):
                nc.vector.tensor_copy(w_sb[:, :, d], evT_sb[:, d, :])
            nc.vector.tensor_tensor(
                w_sb[:, :, :D],
                w_sb[:, :, :D],
                maskT_sb[:, :, None].to_broadcast([K, QSH, D]),
                mybir.AluOpType.mult,
            )

            # ---- main loop over q groups; the last group runs in 4-q halves
            # so its MM/copy/DMA chain overlaps instead of serializing ----
            for g in range(NG):
                ef_t = ef_slots[g]
                halves = (QB // 4) if g == NG - 1 else 1
                ps = pout.tile([128, 2 * C], F32, tag="opsum", name="ps")
                o_sb = outpool.tile([128, 2 * C], F32, tag="osb", name="o_sb")
                for hv in range(halves):
                    js = range(hv * QB // halves, (hv + 1) * QB // halves)
                    nc.vector.tensor_tensor(
                        ef_t[:, js.start : js.stop, :],
                        ef_t[:, js.start : js.stop, :],
                        h_sb[:, None, :].to_broadcast([K, len(js), C]),
                        mybir.AluOpType.mult,
                    )
                    for j in js:
                        f, s = j // 4, j % 4
                        q = g * QB + j
                        nc.tensor.matmul(
                            ps[ds(32 * s, 32), ds(C * f, C)],
                            w_sb[:, q, :],
                            ef_t[:, j, :],
                            start=True,
                            stop=True,
                            tile_position=(0, 32 * s),
                        )
                    nc.scalar.copy(
                        out=o_sb[:, ds(hv * (2 * C) // halves, (2 * C) // halves)],
                        in_=ps[:, ds(hv * (2 * C) // halves, (2 * C) // halves)],
                    )
                # compact scatter: src rows 32s..32s+2 are (d, f, c); dst picks
                # q = g*8 + f*4 + s.  3 descriptors of 2KB each.
                for s in range(4):
                    src_ap = o_sb[ds(32 * s, D), :].rearrange("d (f c) -> d f c", c=C)
                    dst_ap = out_d[g * QB + s : (g + 1) * QB : 4].rearrange(
                        "f d c -> d f c"
                    )
                    eng = (nc.gpsimd, nc.gpsimd, nc.sync, nc.scalar)[s]
                    eng.dma_start(dst_ap, src_ap)

    return _split_multiwaits(nc) if split else nc


def _get_nc():
    if "nc" not in _NC_CACHE:
        _NC_CACHE["nc"] = _build_nc()
    return _NC_CACHE["nc"]


def _in_maps(inputs):
    x = np.asarray(inputs["x"], dtype=np.float32)
    ev = np.asarray(inputs["ev"], dtype=np.float32)
    ef = np.asarray(inputs["ef"], dtype=np.float32)
    am = np.asarray(inputs["access_mask"], dtype=np.float32)
    W1 = np.ascontiguousarray(np.asarray(inputs["W1"], dtype=np.float32))
    b1 = np.ascontiguousarray(np.asarray(inputs["b1"], dtype=np.float32))
    W2 = np.ascontiguousarray(np.asarray(inputs["W2"], dtype=np.float32))
    b2 = np.ascontiguousarray(np.asarray(inputs["b2"], dtype=np.float32))

    maps = []
    for core in range(N_CORES):
        b, qh = core // 2, core % 2
        sl = slice(qh * QSH, (qh + 1) * QSH)
        maps.append(
            {
                "efT": np.ascontiguousarray(ef[b, sl].transpose(1, 0, 2)),
                "evT": np.ascontiguousarray(ev[b, sl].transpose(1, 2, 0)),
                "maskT": np.ascontiguousarray(am[b, sl].T),
                "xT": np.ascontiguousarray(x[b].T),
                "W1": W1,
                "b1": b1,
                "W2": W2,
                "b2": b2,
            }
        )
    return maps


def _gather(results):
    out = np.empty((B, Q, D, C), dtype=np.float32)
    for core in range(N_CORES):
        b, qh = core // 2, core % 2
        out[b, qh * QSH : (qh + 1) * QSH] = results[core]["out"]
    return out


def _run(inputs, trace=False, **kwargs):
    nc = _get_nc()
    res = run_bass_kernel_spmd(
        nc, _in_maps(inputs), list(range(N_CORES)), trace=trace, **kwargs
    )
    return _gather(res.results), res


def kernel(**inputs) -> np.ndarray:
    out, _ = _run(inputs, trace=False)
    return out



# revision 2
# speedup vs baseline: 1.1002x; 1.1002x over previous
"""Trainium2 Bass kernel for the fused GNN message-passing block.

Reference computation (per batch b):
    h = silu(x @ W1 + b1) @ W2 + b2                       # [K, C]
    out[q, d, c] = sum_k mask[q,k] * ev[q,k,d] * ef[q,k,c] * h[k,c]

Sharding: data-parallel over (b, q-half) -> 8 cores, each core handles
one b (of 4) and 64 of the 128 q values.  The large per-q tensors
(ef, ev, mask) are staged bf16 on the host (official gate is
rel_err < 2e-2; bf16 staging lands ~1e-3) which halves the dominant
HBM stream; the MLP runs fp32 and h is cast to bf16 once.

Per-core device program (memory-bound; the ef slice is 4 MiB bf16):
  - DMA descriptor-gen is ~0.65us of serial sequencer time per
    dma_start, so the 7 constant loads go on the gpsimd (SWDGE) queue
    while the 8 ef chunk loads stream on the sync (HWDGE) queue.
  - MLP via PE matmuls with biases folded in as rank-1 matmuls; silu
    is a single fused ACT op per half; h cast fp32->bf16 in the PSUM
    drain.
  - w[k, q, 3] = (mask * ev)^T built by 3 DVE copies + 1 broadcast
    multiply (bf16).
  - main loop per 8-q chunk: DVE multiplies ef by h (broadcast over q,
    bf16 2x mode), then one tiny matmul per q with a 3-column
    stationary w => PSUM rows 32*s+d via tile_position col-groups.
    One ACT copy drains each chunk into a persistent o_all staging
    tile; 4 DMAs at the end write o_all to DRAM (host un-permutes).

The walrus build in this container accepts at most ONE sync wait per
instruction; _split_multiwaits() post-processes the finalized BIR to
hoist extra waits onto single-wait NOPs (see baseline notes).
"""

import numpy as np
import ml_dtypes

import concourse.bass as bass
import concourse.mybir as mybir
import concourse.tile as tile
from concourse.bass import ds, ts
from concourse.bass_utils import run_bass_kernel_spmd

B, Q, K, D, C = 4, 128, 128, 3, 256
N_CORES = 8
QSH = Q // 2  # 64 q rows per core
QB = 8  # q values per ef chunk
NG = QSH // QB  # 8 chunks
F32 = mybir.dt.float32
BF16 = mybir.dt.bfloat16

_NC_CACHE = {}


def _split_multiwaits(nc):
    """Legalize for the 1-sync-wait-per-instruction walrus: hoist all but
    the last wait of each instruction onto single-wait NOPs placed just
    before it on the same engine queue."""
    n = 0
    for f in nc.m.functions:
        for bb in f.blocks:
            out = []
            for inst in bb.instructions:
                si = inst.sync_info
                if si is not None and si.on_wait and len(si.on_wait) > 1:
                    waits = list(si.on_wait)
                    for w in waits[:-1]:
                        n += 1
                        nop = mybir.InstNoOp(
                            name=f"{inst.name}-wsplit{n}", ins=[], outs=[]
                        )
                        nop.engine = inst.engine
                        nop.sync_info = mybir.SyncInfo(on_wait=[w], on_update=[])
                        out.append(nop)
                    inst.sync_info = mybir.SyncInfo(
                        on_wait=[waits[-1]], on_update=list(si.on_update)
                    )
                out.append(inst)
            bb.instructions = out
    return nc


def _build_nc(split=True):
    nc = bass.Bass()

    ef_d = nc.declare_dram_parameter("efT", [K, QSH, C], BF16, isOutput=False)
    evT_d = nc.declare_dram_parameter("evT", [K, D, QSH], BF16, isOutput=False)
    maskT_d = nc.declare_dram_parameter("maskT", [K, QSH], BF16, isOutput=False)
    xT_d = nc.declare_dram_parameter("xT", [C, K], F32, isOutput=False)
    w1_d = nc.declare_dram_parameter("W1", [C, C], F32, isOutput=False)
    b1_d = nc.declare_dram_parameter("b1", [C], F32, isOutput=False)
    w2_d = nc.declare_dram_parameter("W2", [C, C], F32, isOutput=False)
    b2_d = nc.declare_dram_parameter("b2", [C], F32, isOutput=False)
    out_d = nc.declare_dram_parameter("out", [4 * D, NG * 2 * C], F32, isOutput=True)

    with tile.TileContext(nc) as tc:
        with (
            tc.tile_pool(name="const", bufs=1) as cpool,
            tc.tile_pool(name="efp", bufs=1) as efpool,
            tc.tile_pool(name="outp", bufs=1) as outpool,
            tc.tile_pool(name="pprep", bufs=1, space="PSUM") as pprep,
            tc.tile_pool(name="pout", bufs=4, space="PSUM") as pout,
        ):
            # ---- SWDGE (gpsimd) queue: ones memset + the 7 constant loads,
            # in need-order, so the sync queue is free for the ef stream ----
            ones_sb = cpool.tile([1, 128], F32)
            nc.gpsimd.memset(ones_sb[:], 1.0)
            w1_sb = cpool.tile([128, 2, C], F32)
            nc.gpsimd.dma_start(w1_sb[:], w1_d[:, :].rearrange("(o p) n -> p o n", p=128))
            xT_sb = cpool.tile([128, 2, K], F32)
            nc.gpsimd.dma_start(xT_sb[:], xT_d[:, :].rearrange("(o p) k -> p o k", p=128))
            b1_sb = cpool.tile([1, C], F32)
            nc.gpsimd.dma_start(b1_sb[:], b1_d[:][None])
            w2_sb = cpool.tile([128, 2, C], F32)
            nc.gpsimd.dma_start(w2_sb[:], w2_d[:, :].rearrange("(o p) n -> p o n", p=128))
            b2_sb = cpool.tile([1, C], F32)
            nc.gpsimd.dma_start(b2_sb[:], b2_d[:][None])
            evT_sb = cpool.tile([K, D, QSH], BF16)
            nc.gpsimd.dma_start(evT_sb[:], evT_d[:, :, :])
            maskT_sb = cpool.tile([K, QSH], BF16)
            nc.gpsimd.dma_start(maskT_sb[:], maskT_d[:, :])

            # ---- HWDGE (sync) queue: the ef chunk stream ----
            ef_slots = [
                efpool.tile([K, QB, C], BF16, tag=f"ef{g}", name=f"ef{g}")
                for g in range(NG)
            ]
            for g in range(NG):
                nc.sync.dma_start(ef_slots[g][:], ef_d[:, ts(g, QB), :])

            # ---- PE warm-up on an uninitialized scratch tile: flips HAM
            # toward 8/8 while the constant DMAs land.  Results discarded. ----
            w_warm = cpool.tile([128, 512], F32)
            warm_ps = pout.tile([128, 512], F32, tag="opsum", name="warm_ps")
            nc.gpsimd.memset(w_warm[:, :128], 0.0)
            for _ in range(4):
                nc.tensor.matmul(
                    warm_ps[:, :C], w_warm[:, :128], w_warm[:, :C], start=True, stop=True
                )

            # ---- MLP, transpose-free, silu fused on ACT:
            # h1T[d,k] accumulated in PSUM, silu straight from PSUM ----
            h1T_ps = [
                pprep.tile([128, 128], F32, tag=f"prep{i}", name=f"h1T{i}")
                for i in range(2)
            ]
            for dh in range(2):
                nc.tensor.matmul(
                    h1T_ps[dh][:], w1_sb[:, 0, ts(dh, 128)], xT_sb[:, 0, :],
                    start=True, stop=False,
                )
                nc.tensor.matmul(
                    h1T_ps[dh][:], w1_sb[:, 1, ts(dh, 128)], xT_sb[:, 1, :],
                    start=False, stop=False,
                )
                nc.tensor.matmul(
                    h1T_ps[dh][:], b1_sb[:, ts(dh, 128)], ones_sb[:],
                    start=False, stop=True,
                )
            h1sT_sb = cpool.tile([128, 2, 128], F32)
            for dh in range(2):
                nc.scalar.activation(
                    h1sT_sb[:, dh], h1T_ps[dh][:], mybir.ActivationFunctionType.Silu
                )
            h_ps = pprep.tile([128, C], F32, tag="hps", name="h_ps")
            nc.tensor.matmul(h_ps[:], h1sT_sb[:, 0], w2_sb[:, 0], start=True, stop=False)
            nc.tensor.matmul(h_ps[:], h1sT_sb[:, 1], w2_sb[:, 1], start=False, stop=False)
            nc.tensor.matmul(h_ps[:], ones_sb[:], b2_sb[:], start=False, stop=True)
            h_bf = cpool.tile([128, C], BF16)
            nc.scalar.copy(out=h_bf[:], in_=h_ps[:])

            # ---- w[k, q, 3] = (mask * ev)^T, bf16 ----
            w_sb = cpool.tile([128, QSH, D], BF16)
            for d in range(D):
                nc.vector.tensor_copy(w_sb[:, :, d], evT_sb[:, d, :])
            nc.vector.tensor_tensor(
                w_sb[:, :, :],
                w_sb[:, :, :],
                maskT_sb[:, :, None].to_broadcast([K, QSH, D]),
                mybir.AluOpType.mult,
            )

            # ---- main loop over 8-q chunks; all 64 q outputs staged in
            # o_all, written out by 4 DMAs at the end ----
            o_all = outpool.tile([128, NG * 2 * C], F32)
            for g in range(NG):
                ef_t = ef_slots[g]
                halves = 2 if g == NG - 1 else 1
                ps = pout.tile([128, 2 * C], F32, tag="opsum", name="ps")
                for hv in range(halves):
                    js = range(hv * QB // halves, (hv + 1) * QB // halves)
                    nc.vector.tensor_tensor(
                        ef_t[:, js.start : js.stop, :],
                        ef_t[:, js.start : js.stop, :],
                        h_bf[:, None, :].to_broadcast([K, len(js), C]),
                        mybir.AluOpType.mult,
                    )
                    for j in js:
                        f, s = j // 4, j % 4
                        q = g * QB + j
                        nc.tensor.matmul(
                            ps[ds(32 * s, D), ds(C * f, C)],
                            w_sb[:, q, :],
                            ef_t[:, j, :],
                            start=True,
                            stop=True,
                            tile_position=(0, 32 * s),
                        )
                    nc.scalar.copy(
                        out=o_all[
                            :,
                            ds(g * 2 * C + hv * (2 * C) // halves, (2 * C) // halves),
                        ],
                        in_=ps[:, ds(hv * (2 * C) // halves, (2 * C) // halves)],
                    )
            # ---- 4 end-DMAs: PSUM row 32*s+d holds (d, all-q, c) for the
            # s-th q residue; host un-permutes ----
            for s in range(4):
                eng = (nc.sync, nc.scalar, nc.sync, nc.scalar)[s]
                eng.dma_start(out_d[3 * s : 3 * s + 3, :], o_all[ds(32 * s, D), :])

    return _split_multiwaits(nc) if split else nc


def _get_nc():
    if "nc" not in _NC_CACHE:
        _NC_CACHE["nc"] = _build_nc()
    return _NC_CACHE["nc"]


def _in_maps(inputs):
    x = np.asarray(inputs["x"], dtype=np.float32)
    ev = np.asarray(inputs["ev"], dtype=np.float32)
    ef = np.asarray(inputs["ef"], dtype=np.float32)
    am = np.asarray(inputs["access_mask"], dtype=np.float32)
    W1 = np.ascontiguousarray(np.asarray(inputs["W1"], dtype=np.float32))
    b1 = np.ascontiguousarray(np.asarray(inputs["b1"], dtype=np.float32))
    W2 = np.ascontiguousarray(np.asarray(inputs["W2"], dtype=np.float32))
    b2 = np.ascontiguousarray(np.asarray(inputs["b2"], dtype=np.float32))
    bf = ml_dtypes.bfloat16

    maps = []
    for core in range(N_CORES):
        b, qh = core // 2, core % 2
        sl = slice(qh * QSH, (qh + 1) * QSH)
        maps.append(
            {
                "efT": np.ascontiguousarray(
                    ef[b, sl].transpose(1, 0, 2).astype(bf)
                ),
                "evT": np.ascontiguousarray(
                    ev[b, sl].transpose(1, 2, 0).astype(bf)
                ),
                "maskT": np.ascontiguousarray(am[b, sl].T.astype(bf)),
                "xT": np.ascontiguousarray(x[b].T),
                "W1": W1,
                "b1": b1,
                "W2": W2,
                "b2": b2,
            }
        )
    return maps


def _gather(results):
    out = np.empty((B, Q, D, C), dtype=np.float32)
    for core in range(N_CORES):
        b, qh = core // 2, core % 2
        # out DRAM row 3*s+d, col g*512 + f*256 + c  ->  q = g*8 + f*4 + s
        arr = results[core]["out"].reshape(4, D, NG, 2, C)  # [s, d, g, f, c]
        out[b, qh * QSH : (qh + 1) * QSH] = (
            arr.transpose(2, 3, 0, 1, 4).reshape(QSH, D, C)
        )
    return out


def _run(inputs, trace=False, **kwargs):
    nc = _get_nc()
    res = run_bass_kernel_spmd(
        nc, _in_maps(inputs), list(range(N_CORES)), trace=trace, **kwargs
    )
    return _gather(res.results), res


def kernel(**inputs) -> np.ndarray:
    out, _ = _run(inputs, trace=False)
    return out


# revision 5
# speedup vs baseline: 1.3155x; 1.1956x over previous
"""Trainium2 Bass kernel for the fused GNN message-passing block.

Reference computation (per batch b):
    h = silu(x @ W1 + b1) @ W2 + b2                       # [K, C]
    out[q, d, c] = sum_k mask[q,k] * ev[q,k,d] * ef[q,k,c] * h[k,c]

Sharding: data-parallel over (b, q-half) -> 8 cores, each core handles
one b (of 4) and 64 of the 128 q values.  The large per-q tensors
(ef, ev, mask) are staged bf16 on the host (official gate is
rel_err < 2e-2; bf16 staging lands ~1e-3) which halves the dominant
HBM stream; the MLP runs fp32 and h is cast to bf16 once.

Per-core device program (memory-bound; the ef slice is 4 MiB bf16):
  - DMA descriptor-gen is ~0.65us of serial sequencer time per
    dma_start, so the 7 constant loads go on the gpsimd (SWDGE) queue
    while the 8 ef chunk loads stream on the sync (HWDGE) queue.
  - MLP via PE matmuls with biases folded in as rank-1 matmuls; silu
    is a single fused ACT op per half; h cast fp32->bf16 in the PSUM
    drain.
  - w[k, q, 3] = (mask * ev)^T built by 3 DVE copies + 1 broadcast
    multiply (bf16).
  - main loop per 8-q chunk: DVE multiplies ef by h (broadcast over q,
    bf16 2x mode), then one tiny matmul per q with a 3-column
    stationary w => PSUM rows 32*s+d via tile_position col-groups.
    One ACT copy drains each chunk into a persistent o_all staging
    tile; 4 DMAs at the end write o_all to DRAM (host un-permutes).

The walrus build in this container accepts at most ONE sync wait per
instruction; _split_multiwaits() post-processes the finalized BIR to
hoist extra waits onto single-wait NOPs (see baseline notes).
"""

import numpy as np
import ml_dtypes

import concourse.bass as bass
import concourse.mybir as mybir
import concourse.tile as tile
from concourse.bass import ds, ts
from concourse.bass_utils import run_bass_kernel_spmd

B, Q, K, D, C = 4, 128, 128, 3, 256
N_CORES = 8
QSH = Q // 2  # 64 q rows per core
QB = 8  # q values per ef chunk
NG = QSH // QB  # 8 chunks
F32 = mybir.dt.float32
BF16 = mybir.dt.bfloat16

_NC_CACHE = {}


def _split_multiwaits(nc):
    """Legalize for the 1-sync-wait-per-instruction walrus: hoist all but
    the last wait of each instruction onto single-wait NOPs placed just
    before it on the same engine queue."""
    n = 0
    for f in nc.m.functions:
        for bb in f.blocks:
            out = []
            for inst in bb.instructions:
                si = inst.sync_info
                if si is not None and si.on_wait and len(si.on_wait) > 1:
                    waits = list(si.on_wait)
                    for w in waits[:-1]:
                        n += 1
                        nop = mybir.InstNoOp(
                            name=f"{inst.name}-wsplit{n}", ins=[], outs=[]
                        )
                        nop.engine = inst.engine
                        nop.sync_info = mybir.SyncInfo(on_wait=[w], on_update=[])
                        out.append(nop)
                    inst.sync_info = mybir.SyncInfo(
                        on_wait=[waits[-1]], on_update=list(si.on_update)
                    )
                out.append(inst)
            bb.instructions = out
    return nc


def _build_nc(split=True):
    nc = bass.Bass()

    ef_d = nc.declare_dram_parameter("efT", [K, QSH, C], BF16, isOutput=False)
    evT_d = nc.declare_dram_parameter("evT", [K, D, QSH], BF16, isOutput=False)
    maskT_d = nc.declare_dram_parameter("maskT", [K, QSH], BF16, isOutput=False)
    xT_d = nc.declare_dram_parameter("xT", [C, K], F32, isOutput=False)
    w1_d = nc.declare_dram_parameter("W1", [C, C], F32, isOutput=False)
    b1_d = nc.declare_dram_parameter("b1", [C], F32, isOutput=False)
    w2_d = nc.declare_dram_parameter("W2", [C, C], F32, isOutput=False)
    b2_d = nc.declare_dram_parameter("b2", [C], F32, isOutput=False)
    out_d = nc.declare_dram_parameter("out", [4 * D, NG * 2 * C], F32, isOutput=True)

    with tile.TileContext(nc) as tc:
        with (
            tc.tile_pool(name="const", bufs=1) as cpool,
            tc.tile_pool(name="efp", bufs=1) as efpool,
            tc.tile_pool(name="outp", bufs=1) as outpool,
            tc.tile_pool(name="pprep", bufs=1, space="PSUM") as pprep,
            tc.tile_pool(name="pout", bufs=4, space="PSUM") as pout,
        ):
            # ---- HWDGE (scalar) queue: the 7 constant loads in need-order,
            # so the sync queue is free for the ef stream ----
            ones_sb = cpool.tile([1, 128], F32)
            nc.gpsimd.memset(ones_sb[:], 1.0)
            w1_sb = cpool.tile([128, 2, C], F32)
            nc.scalar.dma_start(w1_sb[:], w1_d[:, :].rearrange("(o p) n -> p o n", p=128))
            xT_sb = cpool.tile([128, 2, K], F32)
            nc.scalar.dma_start(xT_sb[:], xT_d[:, :].rearrange("(o p) k -> p o k", p=128))
            b1_sb = cpool.tile([1, C], F32)
            nc.scalar.dma_start(b1_sb[:], b1_d[:][None])
            evT_sb = cpool.tile([K, D, QSH], BF16)
            nc.scalar.dma_start(evT_sb[:], evT_d[:, :, :])
            maskT_sb = cpool.tile([K, QSH], BF16)
            nc.scalar.dma_start(maskT_sb[:], maskT_d[:, :])
            w2_sb = cpool.tile([128, 2, C], F32)
            nc.scalar.dma_start(w2_sb[:], w2_d[:, :].rearrange("(o p) n -> p o n", p=128))
            b2_sb = cpool.tile([1, C], F32)
            nc.scalar.dma_start(b2_sb[:], b2_d[:][None])

            # ---- HWDGE (sync) queue: the ef chunk stream ----
            ef_slots = [
                efpool.tile([K, QB, C], BF16, tag=f"ef{g}", name=f"ef{g}")
                for g in range(NG)
            ]
            for g in range(NG):
                nc.sync.dma_start(ef_slots[g][:], ef_d[:, ts(g, QB), :])

            # ---- PE warm-up: flips HAM toward 8/8 while the constant DMAs
            # land.  Results discarded; memset on the otherwise-idle DVE. ----
            w_warm = cpool.tile([128, 512], F32)
            warm_ps = pout.tile([128, 512], F32, tag="opsum", name="warm_ps")
            nc.vector.memset(w_warm[:], 0.0)
            for _ in range(4):
                nc.tensor.matmul(
                    warm_ps[:, :C], w_warm[:, :128], w_warm[:, :C], start=True, stop=True
                )

            # ---- MLP, transpose-free, silu fused on ACT:
            # h1T[d,k] accumulated in PSUM, silu straight from PSUM ----
            h1T_ps = [
                pprep.tile([128, 128], F32, tag=f"prep{i}", name=f"h1T{i}")
                for i in range(2)
            ]
            for dh in range(2):
                nc.tensor.matmul(
                    h1T_ps[dh][:], w1_sb[:, 0, ts(dh, 128)], xT_sb[:, 0, :],
                    start=True, stop=False,
                )
                nc.tensor.matmul(
                    h1T_ps[dh][:], w1_sb[:, 1, ts(dh, 128)], xT_sb[:, 1, :],
                    start=False, stop=False,
                )
                nc.tensor.matmul(
                    h1T_ps[dh][:], b1_sb[:, ts(dh, 128)], ones_sb[:],
                    start=False, stop=True,
                )
            h1sT_sb = cpool.tile([128, 2, 128], F32)
            for dh in range(2):
                nc.scalar.activation(
                    h1sT_sb[:, dh], h1T_ps[dh][:], mybir.ActivationFunctionType.Silu
                )
            h_ps = pprep.tile([128, C], F32, tag="hps", name="h_ps")
            nc.tensor.matmul(h_ps[:], h1sT_sb[:, 0], w2_sb[:, 0], start=True, stop=False)
            nc.tensor.matmul(h_ps[:], h1sT_sb[:, 1], w2_sb[:, 1], start=False, stop=False)
            nc.tensor.matmul(h_ps[:], ones_sb[:], b2_sb[:], start=False, stop=True)
            h_bf = cpool.tile([128, C], BF16)
            nc.scalar.copy(out=h_bf[:], in_=h_ps[:])

            # ---- w[k, q, 3] = (mask * ev)^T, bf16, on the otherwise-idle
            # gpsimd engine so the DVE stays free for the ef*h stream ----
            w_sb = cpool.tile([128, QSH, D], BF16)
            for d in range(D):
                nc.gpsimd.tensor_copy(w_sb[:, :, d], evT_sb[:, d, :])
            nc.gpsimd.tensor_tensor(
                w_sb[:, :, :],
                w_sb[:, :, :],
                maskT_sb[:, :, None].to_broadcast([K, QSH, D]),
                mybir.AluOpType.mult,
            )

            # ---- main loop over 8-q chunks; all 64 q outputs staged in
            # o_all, written out by 4 DMAs at the end ----
            o_all = outpool.tile([128, NG * 2 * C], F32)
            for g in range(NG):
                ef_t = ef_slots[g]
                halves = 2 if g == NG - 1 else 1
                ps = pout.tile([128, 2 * C], F32, tag="opsum", name="ps")
                for hv in range(halves):
                    js = range(hv * QB // halves, (hv + 1) * QB // halves)
                    nc.vector.tensor_tensor(
                        ef_t[:, js.start : js.stop, :],
                        ef_t[:, js.start : js.stop, :],
                        h_bf[:, None, :].to_broadcast([K, len(js), C]),
                        mybir.AluOpType.mult,
                    )
                    for j in js:
                        f, s = j // 4, j % 4
                        q = g * QB + j
                        nc.tensor.matmul(
                            ps[ds(32 * s, D), ds(C * f, C)],
                            w_sb[:, q, :],
                            ef_t[:, j, :],
                            start=True,
                            stop=True,
                            tile_position=(0, 32 * s),
                        )
                    nc.scalar.copy(
                        out=o_all[
                            :,
                            ds(g * 2 * C + hv * (2 * C) // halves, (2 * C) // halves),
                        ],
                        in_=ps[:, ds(hv * (2 * C) // halves, (2 * C) // halves)],
                    )
                if g == NG // 2 - 1:
                    # first-half output DMAs overlap the remaining stream
                    # (gpsimd queue is idle by now)
                    half = NG // 2 * 2 * C
                    for s in range(4):
                        nc.gpsimd.dma_start(
                            out_d[3 * s : 3 * s + 3, :half],
                            o_all[ds(32 * s, D), :half],
                        )
            # ---- tail: 4 second-half DMAs, spread across queues ----
            half = NG // 2 * 2 * C
            for s in range(4):
                eng = (nc.sync, nc.scalar, nc.sync, nc.scalar)[s]
                eng.dma_start(
                    out_d[3 * s : 3 * s + 3, half:], o_all[ds(32 * s, D), half:]
                )

    return _split_multiwaits(nc) if split else nc


def _get_nc():
    if "nc" not in _NC_CACHE:
        _NC_CACHE["nc"] = _build_nc()
    return _NC_CACHE["nc"]


def _in_maps(inputs):
    x = np.asarray(inputs["x"], dtype=np.float32)
    ev = np.asarray(inputs["ev"], dtype=np.float32)
    ef = np.asarray(inputs["ef"], dtype=np.float32)
    am = np.asarray(inputs["access_mask"], dtype=np.float32)
    W1 = np.ascontiguousarray(np.asarray(inputs["W1"], dtype=np.float32))
    b1 = np.ascontiguousarray(np.asarray(inputs["b1"], dtype=np.float32))
    W2 = np.ascontiguousarray(np.asarray(inputs["W2"], dtype=np.float32))
    b2 = np.ascontiguousarray(np.asarray(inputs["b2"], dtype=np.float32))
    bf = ml_dtypes.bfloat16

    maps = []
    for core in range(N_CORES):
        b, qh = core // 2, core % 2
        sl = slice(qh * QSH, (qh + 1) * QSH)
        maps.append(
            {
                "efT": np.ascontiguousarray(
                    ef[b, sl].transpose(1, 0, 2).astype(bf)
                ),
                "evT": np.ascontiguousarray(
                    ev[b, sl].transpose(1, 2, 0).astype(bf)
                ),
                "maskT": np.ascontiguousarray(am[b, sl].T.astype(bf)),
                "xT": np.ascontiguousarray(x[b].T),
                "W1": W1,
                "b1": b1,
                "W2": W2,
                "b2": b2,
            }
        )
    return maps


def _gather(results):
    out = np.empty((B, Q, D, C), dtype=np.float32)
    for core in range(N_CORES):
        b, qh = core // 2, core % 2
        # out DRAM row 3*s+d, col g*512 + f*256 + c  ->  q = g*8 + f*4 + s
        arr = results[core]["out"].reshape(4, D, NG, 2, C)  # [s, d, g, f, c]
        out[b, qh * QSH : (qh + 1) * QSH] = (
            arr.transpose(2, 3, 0, 1, 4).reshape(QSH, D, C)
        )
    return out


def _run(inputs, trace=False, **kwargs):
    nc = _get_nc()
    res = run_bass_kernel_spmd(
        nc, _in_maps(inputs), list(range(N_CORES)), trace=trace, **kwargs
    )
    return _gather(res.results), res


def kernel(**inputs) -> np.ndarray:
    out, _ = _run(inputs, trace=False)
    return out


# revision 6
# speedup vs baseline: 1.5053x; 1.1443x over previous
"""Trainium2 Bass kernel for the fused GNN message-passing block.

Reference computation (per batch b):
    h = silu(x @ W1 + b1) @ W2 + b2                       # [K, C]
    out[q, d, c] = sum_k mask[q,k] * ev[q,k,d] * ef[q,k,c] * h[k,c]

Sharding: data-parallel over (b, q-half) -> 8 cores, each core handles
one b (of 4) and 64 of the 128 q values.  The large per-q tensors are
staged bf16 on the host (official gate is rel_err < 2e-2; this lands
~5e-3), halving the dominant HBM stream.

Measured DMA behavior drives the structure: each dma_start costs
~0.65us of serial descriptor-gen on its queue, and transfers progress
roughly in doorbell order with ~1us completion latency each.  So:
  - ALL constants ride in two host-packed blob DMAs on the scalar
    (HWDGE) queue: blobA fp32 [W1 | xT | b1-transposed], blobB bf16
    [W2 | evT | maskT | b2-row0].
  - the 8 ef chunk loads stream alone on the sync (HWDGE) queue.
  - b1 is folded into the Silu activation's per-partition bias, so
    stage 1 of the MLP is 4 matmul instructions (fp32 doubles on PE).
  - stage 2 runs bf16 (h1s, W2, b2 all bf16), PSUM accumulates fp32.
  - w[k, q, 3] = (mask * ev)^T built on the otherwise-idle gpsimd.
  - main loop per 8-q chunk: DVE multiplies ef by h (bf16 2x mode,
    1.13us), one tiny matmul per q (3-col stationary, tile_position
    col-groups) -> PSUM rows 32*s+d, one ACT drain into o_all.
  - outputs leave in 3 waves of 4 DMAs (per q-residue s): waves after
    chunks 2 and 5 on gpsimd overlap the stream; the final wave is
    spread across queues to parallelize completion latency.

The walrus build in this container accepts at most ONE sync wait per
instruction; _split_multiwaits() hoists extra waits onto single-wait
NOPs (sequencer executes waits in queue order, so this is equivalent).
"""

import numpy as np
import ml_dtypes

import concourse.bass as bass
import concourse.mybir as mybir
import concourse.tile as tile
from concourse.bass import ds, ts
from concourse.bass_utils import run_bass_kernel_spmd

B, Q, K, D, C = 4, 128, 128, 3, 256
N_CORES = 8
QSH = Q // 2  # 64 q rows per core
QB = 8  # q values per ef chunk
NG = QSH // QB  # 8 chunks
F32 = mybir.dt.float32
BF16 = mybir.dt.bfloat16

SA = 770  # blobA slots: W1 512 | xT 256 | b1T 2
SB = 1024  # blobB slots: W2 512 | evT 192 | maskT 64 | b2 256 (row 0)

_NC_CACHE = {}


def _split_multiwaits(nc):
    """Legalize for the 1-sync-wait-per-instruction walrus: hoist all but
    the last wait of each instruction onto single-wait NOPs placed just
    before it on the same engine queue."""
    n = 0
    for f in nc.m.functions:
        for bb in f.blocks:
            out = []
            for inst in bb.instructions:
                si = inst.sync_info
                if si is not None and si.on_wait and len(si.on_wait) > 1:
                    waits = list(si.on_wait)
                    for w in waits[:-1]:
                        n += 1
                        nop = mybir.InstNoOp(
                            name=f"{inst.name}-wsplit{n}", ins=[], outs=[]
                        )
                        nop.engine = inst.engine
                        nop.sync_info = mybir.SyncInfo(on_wait=[w], on_update=[])
                        out.append(nop)
                    inst.sync_info = mybir.SyncInfo(
                        on_wait=[waits[-1]], on_update=list(si.on_update)
                    )
                out.append(inst)
            bb.instructions = out
    return nc


def _build_nc(split=True):
    nc = bass.Bass()

    ef_d = nc.declare_dram_parameter("efT", [K, QSH, C], BF16, isOutput=False)
    blobA_d = nc.declare_dram_parameter("blobA", [128, SA], F32, isOutput=False)
    blobB_d = nc.declare_dram_parameter("blobB", [128, SB], BF16, isOutput=False)
    out_d = nc.declare_dram_parameter("out", [4 * D, NG * 2 * C], F32, isOutput=True)

    with tile.TileContext(nc) as tc:
        with (
            tc.tile_pool(name="const", bufs=1) as cpool,
            tc.tile_pool(name="efp", bufs=1) as efpool,
            tc.tile_pool(name="outp", bufs=1) as outpool,
            tc.tile_pool(name="pprep", bufs=1, space="PSUM") as pprep,
            tc.tile_pool(name="pout", bufs=4, space="PSUM") as pout,
        ):
            # ---- HWDGE (scalar) queue: the two constant blobs ----
            ones_sb = cpool.tile([1, 128], BF16)
            nc.gpsimd.memset(ones_sb[:], 1.0)
            blobA = cpool.tile([128, SA], F32)
            nc.scalar.dma_start(blobA[:], blobA_d[:, :])
            blobB = cpool.tile([128, SB], BF16)
            nc.scalar.dma_start(blobB[:], blobB_d[:, :])

            # ---- HWDGE (sync) queue: the ef chunk stream ----
            ef_slots = [
                efpool.tile([K, QB, C], BF16, tag=f"ef{g}", name=f"ef{g}")
                for g in range(NG)
            ]
            for g in range(NG):
                nc.sync.dma_start(ef_slots[g][:], ef_d[:, ts(g, QB), :])

            # ---- PE warm-up (bf16: single instruction per matmul): flips
            # HAM toward 8/8 while the blobs land.  Results discarded. ----
            w_warm = cpool.tile([128, 512], BF16)
            warm_ps = pout.tile([128, 512], F32, tag="opsum", name="warm_ps")
            nc.vector.memset(w_warm[:], 0.0)
            for _ in range(3):
                nc.tensor.matmul(
                    warm_ps[:], w_warm[:, :128], w_warm[:], start=True, stop=True
                )

            # ---- MLP.  Stage 1 fp32: h1T[d, k] = (x @ W1)^T; b1 rides the
            # Silu bias (per-partition, since partitions are d here). ----
            h1T_ps = [
                pprep.tile([128, 128], F32, tag=f"prep{i}", name=f"h1T{i}")
                for i in range(2)
            ]
            for dh in range(2):
                nc.tensor.matmul(
                    h1T_ps[dh][:],
                    blobA[:, ds(0 * 256 + dh * 128, 128)],
                    blobA[:, ds(512 + 0 * 128, 128)],
                    start=True,
                    stop=False,
                )
                nc.tensor.matmul(
                    h1T_ps[dh][:],
                    blobA[:, ds(1 * 256 + dh * 128, 128)],
                    blobA[:, ds(512 + 1 * 128, 128)],
                    start=False,
                    stop=True,
                )
            h1sT_sb = cpool.tile([128, 2, 128], BF16)
            for dh in range(2):
                nc.scalar.activation(
                    h1sT_sb[:, dh],
                    h1T_ps[dh][:],
                    mybir.ActivationFunctionType.Silu,
                    bias=blobA[:, ds(768 + dh, 1)],
                )
            # Stage 2 bf16: h[k, c] = h1s @ W2 + b2 (rank-1 via ones)
            h_ps = pprep.tile([128, C], F32, tag="hps", name="h_ps")
            nc.tensor.matmul(
                h_ps[:], h1sT_sb[:, 0], blobB[:, ds(0, 256)], start=True, stop=False
            )
            nc.tensor.matmul(
                h_ps[:], h1sT_sb[:, 1], blobB[:, ds(256, 256)], start=False, stop=False
            )
            nc.tensor.matmul(
                h_ps[:], ones_sb[:], blobB[0:1, ds(768, 256)], start=False, stop=True
            )
            h_bf = cpool.tile([128, C], BF16)
            nc.scalar.copy(out=h_bf[:], in_=h_ps[:])

            # ---- w[k, q, 3] = (mask * ev)^T, bf16, on the otherwise-idle
            # gpsimd engine so the DVE stays free for the ef*h stream ----
            w_sb = cpool.tile([128, QSH, D], BF16)
            for d in range(D):
                nc.gpsimd.tensor_copy(w_sb[:, :, d], blobB[:, ds(512 + d * 64, 64)])
            nc.gpsimd.tensor_tensor(
                w_sb[:, :, :],
                w_sb[:, :, :],
                blobB[:, ds(704, 64)][:, :, None].to_broadcast([K, QSH, D]),
                mybir.AluOpType.mult,
            )

            # ---- main loop over 8-q chunks; all 64 q outputs staged in
            # o_all, written out in 3 waves ----
            o_all = outpool.tile([128, NG * 2 * C], F32)
            for g in range(NG):
                ef_t = ef_slots[g]
                halves = 2 if g == NG - 1 else 1
                ps = pout.tile([128, 2 * C], F32, tag="opsum", name="ps")
                for hv in range(halves):
                    js = range(hv * QB // halves, (hv + 1) * QB // halves)
                    nc.vector.tensor_tensor(
                        ef_t[:, js.start : js.stop, :],
                        ef_t[:, js.start : js.stop, :],
                        h_bf[:, None, :].to_broadcast([K, len(js), C]),
                        mybir.AluOpType.mult,
                    )
                    for j in js:
                        f, s = j // 4, j % 4
                        q = g * QB + j
                        nc.tensor.matmul(
                            ps[ds(32 * s, D), ds(C * f, C)],
                            w_sb[:, q, :],
                            ef_t[:, j, :],
                            start=True,
                            stop=True,
                            tile_position=(0, 32 * s),
                        )
                    nc.scalar.copy(
                        out=o_all[
                            :,
                            ds(g * 2 * C + hv * (2 * C) // halves, (2 * C) // halves),
                        ],
                        in_=ps[:, ds(hv * (2 * C) // halves, (2 * C) // halves)],
                    )
                if g in (2, 5):
                    # overlapped output waves on the idle gpsimd queue
                    lo = 0 if g == 2 else 3 * 2 * C
                    hi = (g + 1) * 2 * C
                    for s in range(4):
                        nc.gpsimd.dma_start(
                            out_d[3 * s : 3 * s + 3, lo:hi],
                            o_all[ds(32 * s, D), lo:hi],
                        )
            # ---- final wave, spread across queues ----
            lo = 6 * 2 * C
            for s in range(4):
                eng = (nc.sync, nc.scalar, nc.gpsimd, nc.gpsimd)[s]
                eng.dma_start(out_d[3 * s : 3 * s + 3, lo:], o_all[ds(32 * s, D), lo:])

    return _split_multiwaits(nc) if split else nc


def _get_nc():
    if "nc" not in _NC_CACHE:
        _NC_CACHE["nc"] = _build_nc()
    return _NC_CACHE["nc"]


def _in_maps(inputs):
    x = np.asarray(inputs["x"], dtype=np.float32)
    ev = np.asarray(inputs["ev"], dtype=np.float32)
    ef = np.asarray(inputs["ef"], dtype=np.float32)
    am = np.asarray(inputs["access_mask"], dtype=np.float32)
    W1 = np.asarray(inputs["W1"], dtype=np.float32)
    b1 = np.asarray(inputs["b1"], dtype=np.float32)
    W2 = np.asarray(inputs["W2"], dtype=np.float32)
    b2 = np.asarray(inputs["b2"], dtype=np.float32)
    bf = ml_dtypes.bfloat16

    blobA = np.zeros((128, SA), dtype=np.float32)
    for o in range(2):
        blobA[:, o * 256 : (o + 1) * 256] = W1[o * 128 : (o + 1) * 128, :]
    maps = []
    for core in range(N_CORES):
        b, qh = core // 2, core % 2
        sl = slice(qh * QSH, (qh + 1) * QSH)
        bA = blobA.copy()
        xT = x[b].T  # [C, K]
        for o in range(2):
            bA[:, 512 + o * 128 : 512 + (o + 1) * 128] = xT[o * 128 : (o + 1) * 128, :]
            bA[:, 768 + o] = b1[o * 128 : (o + 1) * 128]
        bB = np.zeros((128, SB), dtype=bf)
        for o in range(2):
            bB[:, o * 256 : (o + 1) * 256] = W2[o * 128 : (o + 1) * 128, :].astype(bf)
        evT = ev[b, sl].transpose(1, 2, 0)  # [K, D, QSH]
        for d in range(D):
            bB[:, 512 + d * 64 : 512 + (d + 1) * 64] = evT[:, d, :].astype(bf)
        bB[:, 704:768] = am[b, sl].T.astype(bf)
        bB[0, 768:1024] = b2.astype(bf)
        maps.append(
            {
                "efT": np.ascontiguousarray(ef[b, sl].transpose(1, 0, 2).astype(bf)),
                "blobA": bA,
                "blobB": bB,
            }
        )
    return maps


def _gather(results):
    out = np.empty((B, Q, D, C), dtype=np.float32)
    for core in range(N_CORES):
        b, qh = core // 2, core % 2
        # out DRAM row 3*s+d, col g*512 + f*256 + c  ->  q = g*8 + f*4 + s
        arr = results[core]["out"].reshape(4, D, NG, 2, C)  # [s, d, g, f, c]
        out[b, qh * QSH : (qh + 1) * QSH] = (
            arr.transpose(2, 3, 0, 1, 4).reshape(QSH, D, C)
        )
    return out


def _run(inputs, trace=False, **kwargs):
    nc = _get_nc()
    res = run_bass_kernel_spmd(
        nc, _in_maps(inputs), list(range(N_CORES)), trace=trace, **kwargs
    )
    return _gather(res.results), res


def kernel(**inputs) -> np.ndarray:
    out, _ = _run(inputs, trace=False)
    return out


# revision 10
# speedup vs baseline: 1.5531x; 1.0318x over previous
"""Trainium2 Bass kernel for the fused GNN message-passing block.

Reference computation (per batch b):
    h = silu(x @ W1 + b1) @ W2 + b2                       # [K, C]
    out[q, d, c] = sum_k mask[q,k] * ev[q,k,d] * ef[q,k,c] * h[k,c]

Sharding: data-parallel over (b, q-half) -> 8 cores, each core handles
one b (of 4) and 64 of the 128 q values.  The large per-q tensors are
staged bf16 on the host (official gate is rel_err < 2e-2; this lands
~5e-3), halving the dominant HBM stream.

Measured DMA behavior drives the structure: each dma_start costs
~0.65us of serial descriptor-gen on its queue, and transfers progress
roughly in doorbell order with ~1us completion latency each.  So:
  - ALL constants ride in two host-packed blob DMAs on the scalar
    (HWDGE) queue: blobA fp32 [W1 | xT | b1-transposed], blobB bf16
    [W2 | evT | maskT | b2-row0].
  - the 8 ef chunk loads stream alone on the sync (HWDGE) queue.
  - b1 is folded into the Silu activation's per-partition bias, so
    stage 1 of the MLP is 4 matmul instructions (fp32 doubles on PE).
  - stage 2 runs bf16 (h1s, W2, b2 all bf16), PSUM accumulates fp32.
  - w[k, q, 3] = (mask * ev)^T built on the otherwise-idle gpsimd.
  - main loop per 8-q chunk: DVE multiplies ef by h (bf16 2x mode,
    1.13us), one tiny matmul per q (3-col stationary, tile_position
    col-groups) -> PSUM rows 32*s+d, one ACT drain into o_all.
  - outputs leave in 3 waves of 4 DMAs (per q-residue s): waves after
    chunks 2 and 5 on gpsimd overlap the stream; the final wave is
    spread across queues to parallelize completion latency.

The walrus build in this container accepts at most ONE sync wait per
instruction; _split_multiwaits() hoists extra waits onto single-wait
NOPs (sequencer executes waits in queue order, so this is equivalent).
"""

import numpy as np
import ml_dtypes

import concourse.bass as bass
import concourse.mybir as mybir
import concourse.tile as tile
from concourse.bass import ds, ts
from concourse.bass_utils import run_bass_kernel_spmd

B, Q, K, D, C = 4, 128, 128, 3, 256
N_CORES = 8
QSH = Q // 2  # 64 q rows per core
QB = 8  # q values per ef chunk
NG = QSH // QB  # 8 chunks
F32 = mybir.dt.float32
BF16 = mybir.dt.bfloat16

SA = 770  # blobA slots: W1 512 | xT 256 | b1T 2
SB = 1024  # blobB slots: W2 512 | evT 192 | maskT 64 | b2 256 (row 0)

_NC_CACHE = {}


def _split_multiwaits(nc):
    """Legalize for the 1-sync-wait-per-instruction walrus: hoist all but
    the last wait of each instruction onto single-wait NOPs placed just
    before it on the same engine queue."""
    n = 0
    for f in nc.m.functions:
        for bb in f.blocks:
            out = []
            for inst in bb.instructions:
                si = inst.sync_info
                if si is not None and si.on_wait and len(si.on_wait) > 1:
                    waits = list(si.on_wait)
                    for w in waits[:-1]:
                        n += 1
                        nop = mybir.InstNoOp(
                            name=f"{inst.name}-wsplit{n}", ins=[], outs=[]
                        )
                        nop.engine = inst.engine
                        nop.sync_info = mybir.SyncInfo(on_wait=[w], on_update=[])
                        out.append(nop)
                    inst.sync_info = mybir.SyncInfo(
                        on_wait=[waits[-1]], on_update=list(si.on_update)
                    )
                out.append(inst)
            bb.instructions = out
    return nc


def _build_nc(split=True):
    nc = bass.Bass()

    ef_d = nc.declare_dram_parameter("efT", [K, QSH, C], BF16, isOutput=False)
    blobA_d = nc.declare_dram_parameter("blobA", [128, SA], BF16, isOutput=False)
    blobB_d = nc.declare_dram_parameter("blobB", [128, SB], BF16, isOutput=False)
    out_d = nc.declare_dram_parameter("out", [4 * D, NG * 2 * C], F32, isOutput=True)

    with tile.TileContext(nc) as tc:
        with (
            tc.tile_pool(name="const", bufs=1) as cpool,
            tc.tile_pool(name="efp", bufs=1) as efpool,
            tc.tile_pool(name="outp", bufs=1) as outpool,
            tc.tile_pool(name="pprep", bufs=1, space="PSUM") as pprep,
            tc.tile_pool(name="pout", bufs=4, space="PSUM") as pout,
        ):
            # ---- sync (HWDGE) queue, strict FIFO: blobs first so they
            # land before the ef stream, then the 8 ef chunks ----
            ones_sb = cpool.tile([1, 128], BF16)
            nc.gpsimd.memset(ones_sb[:], 1.0)
            blobA = cpool.tile([128, SA], BF16)
            nc.sync.dma_start(blobA[:], blobA_d[:, :])
            blobB = cpool.tile([128, SB], BF16)
            nc.sync.dma_start(blobB[:], blobB_d[:, :])
            ef_slots = [
                efpool.tile([K, QB, C], BF16, tag=f"ef{g}", name=f"ef{g}")
                for g in range(NG)
            ]
            for g in range(NG):
                nc.sync.dma_start(ef_slots[g][:], ef_d[:, ts(g, QB), :])

            # ---- dummy Silu on scratch: forces the ACT table load to the
            # head of the scalar queue, off the h critical path ----
            scr_out = cpool.tile([1, 128], F32)
            nc.scalar.activation(
                scr_out[:], ones_sb[:], mybir.ActivationFunctionType.Silu
            )

            # ---- MLP.  Stage 1 bf16: h1T[d, k] = (x @ W1)^T; b1 rides the
            # Silu bias (per-partition, since partitions are d here). ----
            h1T_ps = [
                pprep.tile([128, 128], F32, tag=f"prep{i}", name=f"h1T{i}")
                for i in range(2)
            ]
            for dh in range(2):
                nc.tensor.matmul(
                    h1T_ps[dh][:],
                    blobA[:, ds(0 * 256 + dh * 128, 128)],
                    blobA[:, ds(512 + 0 * 128, 128)],
                    start=True,
                    stop=False,
                )
                nc.tensor.matmul(
                    h1T_ps[dh][:],
                    blobA[:, ds(1 * 256 + dh * 128, 128)],
                    blobA[:, ds(512 + 1 * 128, 128)],
                    start=False,
                    stop=True,
                )
            h1sT_sb = cpool.tile([128, 2, 128], BF16)
            for dh in range(2):
                nc.scalar.activation(
                    h1sT_sb[:, dh],
                    h1T_ps[dh][:],
                    mybir.ActivationFunctionType.Silu,
                    bias=blobA[:, ds(768 + dh, 1)],
                )
            # Stage 2 bf16: h[k, c] = h1s @ W2 + b2 (rank-1 via ones)
            h_ps = pprep.tile([128, C], F32, tag="hps", name="h_ps")
            nc.tensor.matmul(
                h_ps[:], h1sT_sb[:, 0], blobB[:, ds(0, 256)], start=True, stop=False
            )
            nc.tensor.matmul(
                h_ps[:], h1sT_sb[:, 1], blobB[:, ds(256, 256)], start=False, stop=False
            )
            nc.tensor.matmul(
                h_ps[:], ones_sb[:], blobB[0:1, ds(768, 256)], start=False, stop=True
            )
            h_bf = cpool.tile([128, C], BF16)
            nc.scalar.copy(out=h_bf[:], in_=h_ps[:])

            # ---- w[k, q, 3] = (mask * ev)^T, bf16, on the otherwise-idle
            # gpsimd engine so the DVE stays free for the ef*h stream ----
            w_sb = cpool.tile([128, QSH, D], BF16)
            for d in range(D):
                nc.gpsimd.tensor_copy(w_sb[:, :, d], blobB[:, ds(512 + d * 64, 64)])
            nc.gpsimd.tensor_tensor(
                w_sb[:, :, :],
                w_sb[:, :, :],
                blobB[:, ds(704, 64)][:, :, None].to_broadcast([K, QSH, D]),
                mybir.AluOpType.mult,
            )

            # ---- main loop over 8-q chunks; all 64 q outputs staged in
            # o_all, written out in 3 waves ----
            o_all = outpool.tile([128, NG * 2 * C], F32)
            for g in range(NG):
                ef_t = ef_slots[g]
                halves = 2 if g == NG - 1 else 1
                ps = pout.tile([128, 2 * C], F32, tag="opsum", name="ps")
                for hv in range(halves):
                    js = range(hv * QB // halves, (hv + 1) * QB // halves)
                    nc.vector.tensor_tensor(
                        ef_t[:, js.start : js.stop, :],
                        ef_t[:, js.start : js.stop, :],
                        h_bf[:, None, :].to_broadcast([K, len(js), C]),
                        mybir.AluOpType.mult,
                    )
                    for j in js:
                        f, s = j // 4, j % 4
                        q = g * QB + j
                        nc.tensor.matmul(
                            ps[ds(32 * s, D), ds(C * f, C)],
                            w_sb[:, q, :],
                            ef_t[:, j, :],
                            start=True,
                            stop=True,
                            tile_position=(0, 32 * s),
                        )
                    nc.scalar.copy(
                        out=o_all[
                            :,
                            ds(g * 2 * C + hv * (2 * C) // halves, (2 * C) // halves),
                        ],
                        in_=ps[:, ds(hv * (2 * C) // halves, (2 * C) // halves)],
                    )
                if g in (3, 6):
                    # overlapped output waves on the idle gpsimd queue
                    lo = 0 if g == 3 else 4 * 2 * C
                    hi = (g + 1) * 2 * C
                    for s in range(4):
                        nc.gpsimd.dma_start(
                            out_d[3 * s : 3 * s + 3, lo:hi],
                            o_all[ds(32 * s, D), lo:hi],
                        )
            # ---- final wave (last chunk only), on the free HWDGE queues ----
            lo = 7 * 2 * C
            for s in range(4):
                eng = (nc.sync, nc.scalar, nc.sync, nc.scalar)[s]
                eng.dma_start(out_d[3 * s : 3 * s + 3, lo:], o_all[ds(32 * s, D), lo:])

    return _split_multiwaits(nc) if split else nc


def _get_nc():
    if "nc" not in _NC_CACHE:
        _NC_CACHE["nc"] = _build_nc()
    return _NC_CACHE["nc"]


def _in_maps(inputs):
    x = np.asarray(inputs["x"], dtype=np.float32)
    ev = np.asarray(inputs["ev"], dtype=np.float32)
    ef = np.asarray(inputs["ef"], dtype=np.float32)
    am = np.asarray(inputs["access_mask"], dtype=np.float32)
    W1 = np.asarray(inputs["W1"], dtype=np.float32)
    b1 = np.asarray(inputs["b1"], dtype=np.float32)
    W2 = np.asarray(inputs["W2"], dtype=np.float32)
    b2 = np.asarray(inputs["b2"], dtype=np.float32)
    bf = ml_dtypes.bfloat16

    blobA = np.zeros((128, SA), dtype=bf)
    for o in range(2):
        blobA[:, o * 256 : (o + 1) * 256] = W1[o * 128 : (o + 1) * 128, :].astype(bf)
    maps = []
    for core in range(N_CORES):
        b, qh = core // 2, core % 2
        sl = slice(qh * QSH, (qh + 1) * QSH)
        bA = blobA.copy()
        xT = x[b].T  # [C, K]
        for o in range(2):
            bA[:, 512 + o * 128 : 512 + (o + 1) * 128] = xT[
                o * 128 : (o + 1) * 128, :
            ].astype(bf)
            bA[:, 768 + o] = b1[o * 128 : (o + 1) * 128].astype(bf)
        bB = np.zeros((128, SB), dtype=bf)
        for o in range(2):
            bB[:, o * 256 : (o + 1) * 256] = W2[o * 128 : (o + 1) * 128, :].astype(bf)
        evT = ev[b, sl].transpose(1, 2, 0)  # [K, D, QSH]
        for d in range(D):
            bB[:, 512 + d * 64 : 512 + (d + 1) * 64] = evT[:, d, :].astype(bf)
        bB[:, 704:768] = am[b, sl].T.astype(bf)
        bB[0, 768:1024] = b2.astype(bf)
        maps.append(
            {
                "efT": np.ascontiguousarray(ef[b, sl].transpose(1, 0, 2).astype(bf)),
                "blobA": bA,
                "blobB": bB,
            }
        )
    return maps


def _gather(results):
    out = np.empty((B, Q, D, C), dtype=np.float32)
    for core in range(N_CORES):
        b, qh = core // 2, core % 2
        # out DRAM row 3*s+d, col g*512 + f*256 + c  ->  q = g*8 + f*4 + s
        arr = results[core]["out"].reshape(4, D, NG, 2, C)  # [s, d, g, f, c]
        out[b, qh * QSH : (qh + 1) * QSH] = (
            arr.transpose(2, 3, 0, 1, 4).reshape(QSH, D, C)
        )
    return out


def _run(inputs, trace=False, **kwargs):
    nc = _get_nc()
    res = run_bass_kernel_spmd(
        nc, _in_maps(inputs), list(range(N_CORES)), trace=trace, **kwargs
    )
    return _gather(res.results), res


def kernel(**inputs) -> np.ndarray:
    out, _ = _run(inputs, trace=False)
    return out


# revision 11
# speedup vs baseline: 1.5667x; 1.0087x over previous
"""Trainium2 Bass kernel for the fused GNN message-passing block.

Reference computation (per batch b):
    h = silu(x @ W1 + b1) @ W2 + b2                       # [K, C]
    out[q, d, c] = sum_k mask[q,k] * ev[q,k,d] * ef[q,k,c] * h[k,c]

Sharding: data-parallel over (b, q-half) -> 8 cores, each core handles
one b (of 4) and 64 of the 128 q values.  The large per-q tensors are
staged bf16 on the host (official gate is rel_err < 2e-2; this lands
~5e-3), halving the dominant HBM stream.

Measured DMA behavior drives the structure: each dma_start costs
~0.65us of serial descriptor-gen on its queue, and transfers progress
roughly in doorbell order with ~1us completion latency each.  So:
  - ALL constants ride in two host-packed blob DMAs on the scalar
    (HWDGE) queue: blobA fp32 [W1 | xT | b1-transposed], blobB bf16
    [W2 | evT | maskT | b2-row0].
  - the 8 ef chunk loads stream alone on the sync (HWDGE) queue.
  - b1 is folded into the Silu activation's per-partition bias, so
    stage 1 of the MLP is 4 matmul instructions (fp32 doubles on PE).
  - stage 2 runs bf16 (h1s, W2, b2 all bf16), PSUM accumulates fp32.
  - w[k, q, 3] = (mask * ev)^T built on the otherwise-idle gpsimd.
  - main loop per 8-q chunk: DVE multiplies ef by h (bf16 2x mode,
    1.13us), one tiny matmul per q (3-col stationary, tile_position
    col-groups) -> PSUM rows 32*s+d, one ACT drain into o_all.
  - outputs leave in 3 waves of 4 DMAs (per q-residue s): waves after
    chunks 2 and 5 on gpsimd overlap the stream; the final wave is
    spread across queues to parallelize completion latency.

The walrus build in this container accepts at most ONE sync wait per
instruction; _split_multiwaits() hoists extra waits onto single-wait
NOPs (sequencer executes waits in queue order, so this is equivalent).
"""

import numpy as np
import ml_dtypes

import concourse.bass as bass
import concourse.mybir as mybir
import concourse.tile as tile
from concourse.bass import ds, ts
from concourse.bass_utils import run_bass_kernel_spmd

B, Q, K, D, C = 4, 128, 128, 3, 256
N_CORES = 8
QSH = Q // 2  # 64 q rows per core
QB = 8  # q values per ef chunk
NG = QSH // QB  # 8 chunks
F32 = mybir.dt.float32
BF16 = mybir.dt.bfloat16

SBLOB = 1794  # W1 512 | xT 256 | b1T 2 | W2 512 | evT 192 | maskT 64 | b2 256 (row 0)

_NC_CACHE = {}


def _split_multiwaits(nc):
    """Legalize for the 1-sync-wait-per-instruction walrus: hoist all but
    the last wait of each instruction onto single-wait NOPs placed just
    before it on the same engine queue."""
    n = 0
    for f in nc.m.functions:
        for bb in f.blocks:
            out = []
            for inst in bb.instructions:
                si = inst.sync_info
                if si is not None and si.on_wait and len(si.on_wait) > 1:
                    waits = list(si.on_wait)
                    for w in waits[:-1]:
                        n += 1
                        nop = mybir.InstNoOp(
                            name=f"{inst.name}-wsplit{n}", ins=[], outs=[]
                        )
                        nop.engine = inst.engine
                        nop.sync_info = mybir.SyncInfo(on_wait=[w], on_update=[])
                        out.append(nop)
                    inst.sync_info = mybir.SyncInfo(
                        on_wait=[waits[-1]], on_update=list(si.on_update)
                    )
                out.append(inst)
            bb.instructions = out
    return nc


def _build_nc(split=True):
    nc = bass.Bass()

    ef_d = nc.declare_dram_parameter("efT", [K, QSH, C], BF16, isOutput=False)
    blob_d = nc.declare_dram_parameter("blob", [128, SBLOB], BF16, isOutput=False)
    out_d = nc.declare_dram_parameter("out", [4 * D, NG * 2 * C], F32, isOutput=True)

    with tile.TileContext(nc) as tc:
        with (
            tc.tile_pool(name="const", bufs=1) as cpool,
            tc.tile_pool(name="efp", bufs=1) as efpool,
            tc.tile_pool(name="outp", bufs=1) as outpool,
            tc.tile_pool(name="pprep", bufs=1, space="PSUM") as pprep,
            tc.tile_pool(name="pout", bufs=4, space="PSUM") as pout,
        ):
            # ---- sync (HWDGE) queue, strict FIFO: blobs first so they
            # land before the ef stream, then the 8 ef chunks ----
            ones_sb = cpool.tile([1, 128], BF16)
            nc.gpsimd.memset(ones_sb[:], 1.0)
            blob = cpool.tile([128, SBLOB], BF16)
            nc.sync.dma_start(blob[:], blob_d[:, :])
            ef_slots = [
                efpool.tile([K, QB, C], BF16, tag=f"ef{g}", name=f"ef{g}")
                for g in range(NG)
            ]
            for g in range(NG):
                nc.sync.dma_start(ef_slots[g][:], ef_d[:, ts(g, QB), :])

            # ---- dummy Silu on scratch: forces the ACT table load to the
            # head of the scalar queue, off the h critical path ----
            scr_out = cpool.tile([1, 128], F32)
            nc.scalar.activation(
                scr_out[:], ones_sb[:], mybir.ActivationFunctionType.Silu
            )

            # ---- MLP.  Stage 1 bf16: h1T[d, k] = (x @ W1)^T; b1 rides the
            # Silu bias (per-partition, since partitions are d here). ----
            h1T_ps = [
                pprep.tile([128, 128], F32, tag=f"prep{i}", name=f"h1T{i}")
                for i in range(2)
            ]
            for dh in range(2):
                nc.tensor.matmul(
                    h1T_ps[dh][:],
                    blob[:, ds(0 * 256 + dh * 128, 128)],
                    blob[:, ds(512 + 0 * 128, 128)],
                    start=True,
                    stop=False,
                )
                nc.tensor.matmul(
                    h1T_ps[dh][:],
                    blob[:, ds(1 * 256 + dh * 128, 128)],
                    blob[:, ds(512 + 1 * 128, 128)],
                    start=False,
                    stop=True,
                )
            h1sT_sb = cpool.tile([128, 2, 128], BF16)
            for dh in range(2):
                nc.scalar.activation(
                    h1sT_sb[:, dh],
                    h1T_ps[dh][:],
                    mybir.ActivationFunctionType.Silu,
                    bias=blob[:, ds(768 + dh, 1)],
                )
            # Stage 2 bf16: h[k, c] = h1s @ W2 + b2 (rank-1 via ones)
            h_ps = pprep.tile([128, C], F32, tag="hps", name="h_ps")
            nc.tensor.matmul(
                h_ps[:], h1sT_sb[:, 0], blob[:, ds(770, 256)], start=True, stop=False
            )
            nc.tensor.matmul(
                h_ps[:], h1sT_sb[:, 1], blob[:, ds(1026, 256)], start=False, stop=False
            )
            nc.tensor.matmul(
                h_ps[:], ones_sb[:], blob[0:1, ds(1538, 256)], start=False, stop=True
            )
            h_bf = cpool.tile([128, C], BF16)
            nc.scalar.copy(out=h_bf[:], in_=h_ps[:])

            # ---- w[k, q, 3] = (mask * ev)^T, bf16, on the otherwise-idle
            # gpsimd engine so the DVE stays free for the ef*h stream ----
            w_sb = cpool.tile([128, QSH, D], BF16)
            for d in range(D):
                nc.gpsimd.tensor_copy(w_sb[:, :, d], blob[:, ds(1282 + d * 64, 64)])
            nc.gpsimd.tensor_tensor(
                w_sb[:, :, :],
                w_sb[:, :, :],
                blob[:, ds(1474, 64)][:, :, None].to_broadcast([K, QSH, D]),
                mybir.AluOpType.mult,
            )

            # ---- main loop over 8-q chunks; all 64 q outputs staged in
            # o_all, written out in 3 waves ----
            o_all = outpool.tile([128, NG * 2 * C], F32)
            for g in range(NG):
                ef_t = ef_slots[g]
                halves = 2 if g == NG - 1 else 1
                ps = pout.tile([128, 2 * C], F32, tag="opsum", name="ps")
                for hv in range(halves):
                    js = range(hv * QB // halves, (hv + 1) * QB // halves)
                    nc.vector.tensor_tensor(
                        ef_t[:, js.start : js.stop, :],
                        ef_t[:, js.start : js.stop, :],
                        h_bf[:, None, :].to_broadcast([K, len(js), C]),
                        mybir.AluOpType.mult,
                    )
                    for j in js:
                        f, s = j // 4, j % 4
                        q = g * QB + j
                        nc.tensor.matmul(
                            ps[ds(32 * s, D), ds(C * f, C)],
                            w_sb[:, q, :],
                            ef_t[:, j, :],
                            start=True,
                            stop=True,
                            tile_position=(0, 32 * s),
                        )
                    nc.scalar.copy(
                        out=o_all[
                            :,
                            ds(g * 2 * C + hv * (2 * C) // halves, (2 * C) // halves),
                        ],
                        in_=ps[:, ds(hv * (2 * C) // halves, (2 * C) // halves)],
                    )
                if g == 4:
                    # overlapped output wave; sync + gpsimd queues are idle
                    hi = (g + 1) * 2 * C
                    for s in range(4):
                        eng = (nc.sync, nc.sync, nc.gpsimd, nc.gpsimd)[s]
                        eng.dma_start(
                            out_d[3 * s : 3 * s + 3, :hi],
                            o_all[ds(32 * s, D), :hi],
                        )
            # ---- final wave (chunks 5-7) ----
            lo = 5 * 2 * C
            for s in range(4):
                eng = (nc.sync, nc.sync, nc.gpsimd, nc.gpsimd)[s]
                eng.dma_start(out_d[3 * s : 3 * s + 3, lo:], o_all[ds(32 * s, D), lo:])

    return _split_multiwaits(nc) if split else nc


def _get_nc():
    if "nc" not in _NC_CACHE:
        _NC_CACHE["nc"] = _build_nc()
    return _NC_CACHE["nc"]


def _in_maps(inputs):
    x = np.asarray(inputs["x"], dtype=np.float32)
    ev = np.asarray(inputs["ev"], dtype=np.float32)
    ef = np.asarray(inputs["ef"], dtype=np.float32)
    am = np.asarray(inputs["access_mask"], dtype=np.float32)
    W1 = np.asarray(inputs["W1"], dtype=np.float32)
    b1 = np.asarray(inputs["b1"], dtype=np.float32)
    W2 = np.asarray(inputs["W2"], dtype=np.float32)
    b2 = np.asarray(inputs["b2"], dtype=np.float32)
    bf = ml_dtypes.bfloat16

    blob0 = np.zeros((128, SBLOB), dtype=bf)
    for o in range(2):
        blob0[:, o * 256 : (o + 1) * 256] = W1[o * 128 : (o + 1) * 128, :].astype(bf)
        blob0[:, 770 + o * 256 : 770 + (o + 1) * 256] = W2[
            o * 128 : (o + 1) * 128, :
        ].astype(bf)
    blob0[0, 1538:1794] = b2.astype(bf)
    maps = []
    for core in range(N_CORES):
        b, qh = core // 2, core % 2
        sl = slice(qh * QSH, (qh + 1) * QSH)
        bb = blob0.copy()
        xT = x[b].T  # [C, K]
        for o in range(2):
            bb[:, 512 + o * 128 : 512 + (o + 1) * 128] = xT[
                o * 128 : (o + 1) * 128, :
            ].astype(bf)
            bb[:, 768 + o] = b1[o * 128 : (o + 1) * 128].astype(bf)
        evT = ev[b, sl].transpose(1, 2, 0)  # [K, D, QSH]
        for d in range(D):
            bb[:, 1282 + d * 64 : 1282 + (d + 1) * 64] = evT[:, d, :].astype(bf)
        bb[:, 1474:1538] = am[b, sl].T.astype(bf)
        maps.append(
            {
                "efT": np.ascontiguousarray(ef[b, sl].transpose(1, 0, 2).astype(bf)),
                "blob": bb,
            }
        )
    return maps


def _gather(results):
    out = np.empty((B, Q, D, C), dtype=np.float32)
    for core in range(N_CORES):
        b, qh = core // 2, core % 2
        # out DRAM row 3*s+d, col g*512 + f*256 + c  ->  q = g*8 + f*4 + s
        arr = results[core]["out"].reshape(4, D, NG, 2, C)  # [s, d, g, f, c]
        out[b, qh * QSH : (qh + 1) * QSH] = (
            arr.transpose(2, 3, 0, 1, 4).reshape(QSH, D, C)
        )
    return out


def _run(inputs, trace=False, **kwargs):
    nc = _get_nc()
    res = run_bass_kernel_spmd(
        nc, _in_maps(inputs), list(range(N_CORES)), trace=trace, **kwargs
    )
    return _gather(res.results), res


def kernel(**inputs) -> np.ndarray:
    out, _ = _run(inputs, trace=False)
    return out
